# revision 2
# baseline (speedup 1.0000x reference)
"""MultiHeadCrossAttention kernel for 8 Trainium2 NeuronCores.

Sharding: pure data-parallel over batch (B=8 -> 1 batch element per core).

Design (309,383ns cost-model estimate, from 310,945ns prior / 522,000ns
unoptimized; rel err 2.6e-3):
  - Activations pre-transposed and downcast on the HOST (fp16 q/k/scores
    path, fp8-e4m3 weights for the v and out projections, x16 host scale).
  - v projection and out_proj in fp8 DoubleRow (0.5 cyc/row); q/k
    projections, scores and attn@v in fp16 (1 cyc/row on PE).
  - Scores PSUM is THREE separate [P,2,512] tiles rotating per half-job:
    tile-granular dependency tracking gives exp(m) -> scores(m+3) about
    1.5 jobs of slack (a single ring tile is tracked conservatively and
    serializes exp->scores->exp at ~3.8us/job).
  - softmax denominator via a ones-column in the v operand; reciprocal on
    DVE; partition-broadcast on Pool; exp on Act only (one act-table load).
  - attention-weights mean: fp16 probs, normalize split DVE (kt 0-4) /
    Pool (kt 5-7), two parity accumulation chains on DVE merged per
    q-block, DMA-XBAR transposes to natural [q,k] layout.
  - per-job pipeline emitted b(oldest) -> a -> front so ready work never
    queues behind not-yet-ready work on the in-order engine queues; qk
    projection halves paced by a need-based counter so every kT/qT chunk
    is emitted before the first front that reads it.
  - out_proj chunks drained one per b-step through the single proj psum
    bank; W store path staged across the next q-block's first b-steps.
  - LayerNorm inline per q-block: bn_stats/bn_aggr + rsqrt via uint32 bit
    trick + 1 Newton step on DVE.
"""

import numpy as np
import ml_dtypes
from contextlib import ExitStack

import concourse.bacc as bacc
import concourse.bass as bass
import concourse.tile as tile
from concourse import mybir
from concourse.bass_utils import run_bass_kernel_spmd
from concourse.masks import make_identity

E = 1024
H = 16
DH = 64
L = 1024
P = 128
QB = 256          # q-block size
NQB = L // QB     # 4
NKT = L // P      # 8 k-tiles
NEC = E // P      # 8 feature chunks
VS = H * (DH + 1)  # 1040 v columns per k-tile (65 per head)
LN_EPS = 1e-5

F32 = mybir.dt.float32
FP16 = mybir.dt.float16
E4M3 = mybir.dt.float8e4
AF = mybir.ActivationFunctionType
OP = mybir.AluOpType
DR = mybir.MatmulPerfMode.DoubleRow

NP_FP16 = np.float16
NP_E4M3 = ml_dtypes.float8_e4m3


def _emit(nc, tc, io):
    ctx = tc.ctx
    ctx.enter_context(nc.allow_low_precision("fp8 attention"))

    const = ctx.enter_context(tc.tile_pool(name="const", bufs=1))
    persist = ctx.enter_context(tc.tile_pool(name="persist", bufs=1))

    ones1 = const.tile([1, P], FP16)
    nc.vector.memset(ones1[:], 1.0)
    one_u = const.tile([P, 1], mybir.dt.uint32)
    nc.vector.memset(one_u[:], 1)
    magic_u = const.tile([P, 1], mybir.dt.uint32)
    nc.vector.memset(magic_u[:], 0x5F3759DF)
    ident_f = const.tile([P, P], F32)
    make_identity(nc, ident_f[:])
    ident = const.tile([P, P], FP16)
    nc.vector.tensor_copy(ident[:], ident_f[:])
    ident8p = const.tile([P, 2, P], E4M3)
    nc.vector.tensor_copy(ident8p[:, 0, :], ident_f[:])
    nc.vector.tensor_copy(ident8p[:, 1, :], ident_f[:])

    # persistent activations / weights
    qT = persist.tile([P, NEC, L], FP16)     # [e%128, e//128, l]
    kT = persist.tile([P, NEC, L], FP16)
    v_sb = persist.tile([P, NKT * VS], FP16)  # [l%128, kt*(16 heads x 65)]
    wo8 = persist.tile([P, NEC, E], E4M3)    # 16*Wo.T  [e_in, e_out]

    # ones columns (softmax denominator trick)
    nc.vector.memset(
        v_sb[:].rearrange("p (n d) -> p n d", d=DH + 1)[:, :, DH:DH + 1], 1.0,
    )

    ld_pool = ctx.enter_context(tc.tile_pool(name="ld", bufs=1))
    # PSUM: manual slot management. sc 2 banks, av 1 bank, p1 1 bank, W 4.
    psum_all = ctx.enter_context(tc.tile_pool(name="psum", bufs=1, space="PSUM"))
    # 3 separate half-job score tiles: tile-granular dependency tracking gives
    # exp(m) -> scores(m+3) 1.5 jobs of real slack (single-tile rings are
    # tracked conservatively and serialize).
    sc_t = [psum_all.tile([P, 2, 512], F32, name=f"sc_t{i}")
            for i in range(3)]  # 6 banks
    av_sl = psum_all.tile([P, 2, QB], F32)     # 1 bank: 2 slots of [P,256]
    p1_sl = psum_all.tile([P, 512], F32)       # 1 bank: qk/out_proj/W bursts

    expT_pool = ctx.enter_context(tc.tile_pool(name="expT", bufs=6))
    wch_pool = ctx.enter_context(tc.tile_pool(name="wch", bufs=4))
    a8_pool = ctx.enter_context(tc.tile_pool(name="a8", bufs=2))
    invbc_pool = ctx.enter_context(tc.tile_pool(name="invbc", bufs=6))
    wacc_pool = ctx.enter_context(tc.tile_pool(name="wacc", bufs=1))
    wnat_pool = ctx.enter_context(tc.tile_pool(name="wnat", bufs=2))
    xqb_pool = ctx.enter_context(tc.tile_pool(name="xqb", bufs=2))
    small = ctx.enter_context(tc.tile_pool(name="small", bufs=4))
    z_pool = ctx.enter_context(tc.tile_pool(name="z16", bufs=2))
    ysb_pool = ctx.enter_context(tc.tile_pool(name="ysb", bufs=2))

    # ---- input loads, in dependency-criticality order ----
    vw = ld_pool.tile([P, 2 * NEC, E], E4M3, tag="aTx")
    aT_q = ld_pool.tile([P, NEC, L], FP16, tag="aTq")
    wt_q = ld_pool.tile([P, NEC, E], FP16, tag="wtq")
    # consts: one early DMA [1,4096] = [bv(1024)|bo(1024) | gamma | beta]
    crow = const.tile([1, 4 * E], FP16)
    nc.sync.dma_start(out=crow[:], in_=io["consts"][:])
    # vw8 arrives in 4 interleaved groups [w-pair | a-pair]
    for g in range(4):
        nc.sync.dma_start(
            out=vw[:, 4 * g:4 * (g + 1), :],
            in_=io["vw8"][512 * g:512 * (g + 1), :].rearrange(
                "(c p) n -> p c n", p=P),
        )
    bvo_row = crow[:, 0:2 * E]
    g_row = crow[:, 2 * E:3 * E]
    b_row = crow[:, 3 * E:4 * E]
    bqk_col = const.tile([P, 2 * NEC], F32)

    for i in range(2):
        nc.sync.dma_start(
            out=bqk_col[:, NEC * i:NEC * (i + 1)],
            in_=io["bqk"][i, :].rearrange("(m p) -> p m", p=P),
        )
    for g in range(4):
        nc.sync.dma_start(
            out=wt_q[:, 2 * g:2 * g + 2, :],
            in_=io["wq"][256 * g:256 * (g + 1), :].rearrange("(c p) n -> p c n", p=P))
        nc.sync.dma_start(
            out=aT_q[:, 2 * g:2 * g + 2, :],
            in_=io["xT"][256 * g:256 * (g + 1), :].rearrange("(c p) l -> p c l", p=P))
    gamma_bc = const.tile([P, E], FP16)
    beta_bc = const.tile([P, E], FP16)

    # ---- v projection (fp8 DoubleRow, x16 scale) ----
    # psum chains rotate through the scores ring (idle until jobs start) so
    # vproj pipelines against its Act drains and leaves p1 free for qk_proj
    for m in range(NEC):
        for n in range(2):
            i = 2 * m + n
            ps = sc_t[(i // 2) % 3][:, i % 2, :]
            for sub in range(2):
                for pr in range(4):
                    nc.tensor.matmul(
                        ps[:, 256 * sub:256 * (sub + 1)],
                        vw[:, 4 * pr + 2:4 * pr + 4, P * m:P * (m + 1)],
                        vw[:, 4 * pr:4 * pr + 2,
                           512 * n + 256 * sub:512 * n + 256 * (sub + 1)],
                        start=(pr == 0), stop=False,
                        perf_mode=DR,
                    )
                nc.tensor.matmul(
                    ps[:, 256 * sub:256 * (sub + 1)],
                    ones1[0:1, :],
                    bvo_row[:, 512 * n + 256 * sub:512 * n + 256 * (sub + 1)],
                    start=False, stop=True,
                )
            dst = v_sb[:, VS * m + 520 * n:VS * m + 520 * (n + 1)]
            nc.scalar.copy(
                out=dst.rearrange("p (h d) -> p h d", d=DH + 1)[:, :, 0:DH],
                in_=ps.rearrange("p (h d) -> p h d", d=DH),
            )

    nc.gpsimd.partition_broadcast(gamma_bc[:], g_row)
    nc.gpsimd.partition_broadcast(beta_bc[:], b_row)

    # k loads reuse the v buffers (freed by the v projection above)
    aT_k = ld_pool.tile([P, NEC, L], FP16, tag="aTx")
    wt_k = ld_pool.tile([P, NEC, E], FP16, tag="wtx")
    for g in range(4):
        nc.sync.dma_start(
            out=wt_k[:, 2 * g:2 * g + 2, :],
            in_=io["wk"][256 * g:256 * (g + 1), :].rearrange("(c p) n -> p c n", p=P))
        nc.sync.dma_start(
            out=aT_k[:, 2 * g:2 * g + 2, :],
            in_=io["kTa"][256 * g:256 * (g + 1), :].rearrange("(c p) l -> p c l", p=P))
    nc.sync.dma_start(out=wo8[:], in_=io["wo8"].rearrange("(c p) n -> p c n", p=P))

    def qk_proj_half(ti, m, n):
        aT, wt = (aT_q, wt_q) if ti == 0 else (aT_k, wt_k)
        ps = p1_sl[:]
        for c in range(NEC):
            nc.tensor.matmul(
                ps,
                wt[:, c, P * m:P * (m + 1)],
                aT[:, c, 512 * n:512 * (n + 1)],
                start=(c == 0), stop=(c == NEC - 1),
            )
        dst = (qT if ti == 0 else kT)[:, m, 512 * n:512 * (n + 1)]
        nc.scalar.activation(
            dst, ps, AF.Identity,
            bias=bqk_col[:, NEC * ti + m:NEC * ti + m + 1],
        )

    # ---- per-qb state ----
    st = {}

    def qb_begin(qb):
        q0 = QB * qb
        x_qb = xqb_pool.tile([P, 2, E], FP16, tag="xqb", name=f"xqb_{qb}")
        nc.sync.dma_start(
            out=x_qb[:],
            in_=io["xnat"][q0:q0 + QB, :].rearrange("(s p) e -> p s e", p=P),
        )
        st[qb] = dict(
            x_qb=x_qb,
            attnT8=a8_pool.tile([P, NEC, QB], E4M3, tag="attnT8", name=f"a8_{qb}"),
            ysb=ysb_pool.tile([P, 2, E], FP16, tag="ysb", name=f"y_{qb}"),
            Wacc=[wch_pool.tile([P, NKT * QB], FP16, tag="wch",
                                name=f"wa_{qb}_{p}") for p in range(2)],
        )

    qtr = [0]  # global quarter counter (ring position)

    def head_front(qb, h):
        if h == 0:
            qb_begin(qb)
        q0 = QB * qb
        hb = (h % 2) * DH
        hc = h // 2
        expT = expT_pool.tile([P, NKT * QB], FP16, tag="expT",
                              name=f"expT_{qb}_{h}")
        for half in range(2):
            stile = sc_t[qtr[0] % 3]
            qtr[0] += 1
            for j in range(4):
                kt = 4 * half + j
                nc.tensor.matmul(
                    stile[:, j // 2, 256 * (j % 2):256 * (j % 2 + 1)],
                    kT[hb:hb + DH, hc, P * kt:P * (kt + 1)],
                    qT[hb:hb + DH, hc, q0:q0 + QB],
                    start=True, stop=True,
                )
            nc.scalar.activation(
                expT[:, 1024 * half:1024 * (half + 1)],
                stile[:], AF.Exp, scale=0.125,
            )
        return expT

    def tail_a(qb, h, expT):
        s = st[qb]
        hb = (h % 2) * DH
        hc = h // 2
        av = av_sl[:, (H * qb + h) % 2, :]
        for kt in range(NKT):
            nc.tensor.matmul(
                av[0:DH + 1, 0:QB],
                v_sb[:, VS * kt + (DH + 1) * h:VS * kt + (DH + 1) * (h + 1)],
                expT[:, QB * kt:QB * (kt + 1)],
                start=(kt == 0), stop=(kt == NKT - 1),
            )
        inv = small.tile([1, QB], FP16, tag="inv", name=f"inv_{qb}_{h}")
        nc.vector.reciprocal(inv[:], av[DH:DH + 1, 0:QB])
        inv_bc = invbc_pool.tile([P, QB], FP16, tag="invbc", name=f"ib_{qb}_{h}")
        nc.gpsimd.partition_broadcast(inv_bc[:], inv[:])
        nc.vector.tensor_tensor(
            out=s["attnT8"][hb:hb + DH, hc, :],
            in0=av[0:DH, 0:QB], in1=inv_bc[0:DH, :], op=OP.mult,
        )
        return inv_bc

    def tail_b(qb, h, expT, inv_bc):
        s = st[qb]
        iap = inv_bc[:]
        KD = 5  # kt 0..KD-1 on DVE, rest on Pool
        ee = expT[:].rearrange("p (n d) -> p n d", d=QB)
        Wacc = s["Wacc"][h % 2]
        wa = Wacc[:].rearrange("p (n d) -> p n d", d=QB)
        if h <= 1:
            nc.vector.tensor_tensor(
                out=wa[:, 0:KD, :], in0=ee[:, 0:KD, :],
                in1=bass.AP(tensor=iap.tensor, offset=iap.offset,
                            ap=[iap.ap[0], [0, KD], iap.ap[1]]),
                op=OP.mult,
            )
            nc.gpsimd.tensor_tensor(
                out=wa[:, KD:NKT, :], in0=ee[:, KD:NKT, :],
                in1=bass.AP(tensor=iap.tensor, offset=iap.offset,
                            ap=[iap.ap[0], [0, NKT - KD], iap.ap[1]]),
                op=OP.mult,
            )
        else:
            nc.vector.tensor_tensor(
                out=ee[:, 0:KD, :], in0=ee[:, 0:KD, :],
                in1=bass.AP(tensor=iap.tensor, offset=iap.offset,
                            ap=[iap.ap[0], [0, KD], iap.ap[1]]),
                op=OP.mult,
            )
            nc.gpsimd.tensor_tensor(
                out=ee[:, KD:NKT, :], in0=ee[:, KD:NKT, :],
                in1=bass.AP(tensor=iap.tensor, offset=iap.offset,
                            ap=[iap.ap[0], [0, NKT - KD], iap.ap[1]]),
                op=OP.mult,
            )
            nc.vector.tensor_tensor(out=Wacc[:], in0=Wacc[:], in1=expT[:],
                                    op=OP.add)

    # out_proj for one psum chunk (qs, eb); p1 single slot, interleaved
    def op_chunk(qb, qs, eb):
        s = st[qb]
        po = p1_sl[:]
        attnT8 = s["attnT8"]
        for sub in range(2):
            for pr in range(4):
                nc.tensor.matmul(
                    po[:, 256 * sub:256 * (sub + 1)],
                    attnT8[:, 2 * pr:2 * pr + 2, P * qs:P * (qs + 1)],
                    wo8[:, 2 * pr:2 * pr + 2,
                        512 * eb + 256 * sub:512 * eb + 256 * (sub + 1)],
                    start=(pr == 0), stop=False,
                    perf_mode=DR,
                )
            nc.tensor.matmul(
                po[:, 256 * sub:256 * (sub + 1)],
                ones1[0:1, :],
                bvo_row[:, E + 512 * eb + 256 * sub:
                        E + 512 * eb + 256 * (sub + 1)],
                start=False, stop=True,
            )
        nc.vector.scalar_tensor_tensor(
            out=s["ysb"][:, qs, 512 * eb:512 * (eb + 1)],
            in0=po[:], scalar=1.0 / 256.0,
            in1=s["x_qb"][:, qs, 512 * eb:512 * (eb + 1)],
            op0=OP.mult, op1=OP.add,
        )

    def fw_merge(qb):
        W0, W1 = st[qb]["Wacc"]
        nc.vector.tensor_tensor(out=W0[:], in0=W0[:], in1=W1[:], op=OP.add)

    def fw_out(qb, qs):
        q0 = QB * qb
        Wacc = st[qb]["Wacc"][0]
        wnat = wnat_pool.tile([P, NKT, P], FP16, tag="wnat",
                              name=f"wn_{qb}_{qs}")
        for kt in range(NKT):
            nc.sync.dma_start_transpose(
                wnat[:, kt, :],
                Wacc[:, QB * kt + P * qs:QB * kt + P * (qs + 1)],
            )
        nc.sync.dma_start(
            out=io["w16"][q0 + P * qs:q0 + P * (qs + 1), :], in_=wnat[:]
        )
        if qs == 1:
            st[qb]["Wacc"] = []

    def finalize_ln(qb):
        # LayerNorm: batched stats + one-shot rsqrt (bit trick + 1 Newton)
        yqb = st[qb]["ysb"]
        mvs = []
        for qs in range(2):
            t = 2 * qb + qs
            stats = small.tile([P, 2, 6], F32, tag="stats", name=f"st_{t}")
            ychg = yqb[:, qs, :].rearrange("p (s f) -> p s f", f=512)
            for sg in range(2):
                nc.vector.bn_stats(out=stats[:, sg, :], in_=ychg[:, sg, :])
            mv = small.tile([P, 2], F32, tag="mv", name=f"mv_{t}")
            nc.vector.bn_aggr(out=mv[:], in_=stats[:])
            mvs.append(mv)
        ve = small.tile([P, 2], F32, tag="ve", name=f"ve_{qb}")
        for qs in range(2):
            nc.vector.tensor_scalar_add(out=ve[:, qs:qs + 1],
                                        in0=mvs[qs][:, 1:2], scalar1=LN_EPS)
        y0u = small.tile([P, 2], mybir.dt.uint32, tag="y0u", name=f"y0_{qb}")
        ou = bass.AP(tensor=one_u.tensor, offset=one_u[:].offset,
                     ap=[one_u[:].ap[0], [0, 2]])
        mu = bass.AP(tensor=magic_u.tensor, offset=magic_u[:].offset,
                     ap=[magic_u[:].ap[0], [0, 2]])
        nc.vector.tensor_tensor(out=y0u[:], in0=ve[:].bitcast(mybir.dt.uint32),
                                in1=ou, op=OP.logical_shift_right)
        nc.vector.tensor_tensor(out=y0u[:], in0=mu, in1=y0u[:], op=OP.subtract)
        y0 = y0u[:].bitcast(F32)
        rstd = small.tile([P, 2], F32, tag="rstd", name=f"rs_{qb}")
        tmp = small.tile([P, 2], F32, tag="tmp", name=f"tm_{qb}")
        nc.vector.tensor_tensor(out=tmp[:], in0=y0, in1=y0, op=OP.mult)
        nc.vector.tensor_tensor(out=tmp[:], in0=tmp[:], in1=ve[:], op=OP.mult)
        nc.vector.tensor_scalar(out=tmp[:], in0=tmp[:], scalar1=-0.5,
                                scalar2=1.5, op0=OP.mult, op1=OP.add)
        nc.vector.tensor_tensor(out=rstd[:], in0=y0, in1=tmp[:], op=OP.mult)
        for qs in range(2):
            t = 2 * qb + qs
            negmr = small.tile([P, 1], F32, tag="negmr", name=f"nm_{t}")
            nc.vector.tensor_tensor(out=negmr[:], in0=mvs[qs][:, 0:1],
                                    in1=rstd[:, qs:qs + 1], op=OP.mult)
            nc.vector.tensor_scalar_mul(out=negmr[:], in0=negmr[:], scalar1=-1.0)
            z16 = z_pool.tile([P, E], FP16, tag="z16", name=f"z_{t}")
            nc.scalar.activation(z16[:], yqb[:, qs, :], AF.Identity,
                                 bias=negmr[:], scale=rstd[:, qs:qs + 1])
            nc.vector.tensor_tensor(out=z16[:], in0=z16[:], in1=gamma_bc[:],
                                    op=OP.mult)
            nc.vector.tensor_tensor(out=z16[:], in0=z16[:], in1=beta_bc[:],
                                    op=OP.add)
            nc.sync.dma_start(out=io["y16"][P * t:P * (t + 1), :], in_=z16[:])

    # ---- fused pipeline ----
    # qk halves queue: q-proj m0..2 first (6 halves), then interleave rest
    qk_queue = [(0, m, n) for m in range(3) for n in range(2)]
    qk_rest = []
    for m in range(NEC):
        qk_rest.append((1, m, 0))
        qk_rest.append((1, m, 1))
        if m + 3 < NEC:
            qk_rest.append((0, m + 3, 0))
            qk_rest.append((0, m + 3, 1))
    op_queue = []   # pending out_proj chunks

    for t in qk_queue:
        qk_proj_half(*t)

    jobs = [(qb, h) for qb in range(NQB) for h in range(H)]
    N = len(jobs)
    fronts = {}
    invs = {}
    nf = 0
    na = 0
    nb = 0

    def emit_front():
        nonlocal nf
        qb, h = jobs[nf]
        fronts[nf] = head_front(qb, h)
        nf += 1

    def emit_a():
        nonlocal na
        qb, h = jobs[na]
        invs[na] = tail_a(qb, h, fronts[na])
        na += 1
        if h == H - 1:
            for qs in range(2):
                for eb in range(2):
                    op_queue.append((qb, qs, eb))

    def emit_b():
        nonlocal nb
        qb, h = jobs[nb]
        tail_b(qb, h, fronts.pop(nb), invs.pop(nb))
        nb += 1
        if h == H - 1:
            fw_merge(qb)
        elif h == 0 and qb > 0:
            fw_out(qb - 1, 0)
        elif h == 1 and qb > 0:
            fw_out(qb - 1, 1)
        # drain up to 2 out_proj chunks per b-step
        for _ in range(2):
            if op_queue:
                op_chunk(*op_queue.pop(0))

    ln_done = set()

    def maybe_ln():
        # LN(qb) once all 4 out_proj chunks for qb are emitted
        for qb in range(NQB):
            if qb in ln_done:
                continue
            if all((qb, qs, eb) not in op_queue
                   for qs in range(2) for eb in range(2)) \
                    and nb >= H * (qb + 1) + 6:
                ln_done.add(qb)
                finalize_ln(qb)

    # steady pipeline; b (oldest, deps ready) before a before fronts so
    # ready work never queues behind not-yet-ready work on DVE/Pool/PE
    qi = 0
    while nb < N:
        # qk halves must stay ahead of the fronts that read them: fronts for
        # jobs 2i,2i+1 (qb0) need k chunk m=i and q chunk m=i (m>=3 from rest)
        need = 4 * (nf // 2 + 1)
        if nb < na - 1 or (na == N and nb < N):
            emit_b()
        if qi < min(need, len(qk_rest)):
            qk_proj_half(*qk_rest[qi])
            qi += 1
        if nb < na - 1 or (na == N and nb < N):
            emit_b()
        if qi < min(need, len(qk_rest)):
            qk_proj_half(*qk_rest[qi])
            qi += 1
        if na < nf - 1 or (nf == N and na < N):
            emit_a()
        if na < nf - 1 or (nf == N and na < N):
            emit_a()
        if qi < min(need, len(qk_rest)):
            qk_proj_half(*qk_rest[qi])
            qi += 1
        if nf < N:
            emit_front()
        if qi < min(need, len(qk_rest)):
            qk_proj_half(*qk_rest[qi])
            qi += 1
        if nf < N:
            emit_front()
        maybe_ln()
    # tail: release the y16 path (out_proj + LN) early, then the qb3 W quads
    while op_queue:
        op_chunk(*op_queue.pop(0))
    maybe_ln()
    for qb in range(NQB):
        if qb not in ln_done:
            finalize_ln(qb)
    fw_out(NQB - 1, 0)
    fw_out(NQB - 1, 1)


_CACHED = None


def _build():
    global _CACHED
    if _CACHED is not None:
        return _CACHED
    nc = bacc.Bacc("TRN2", target_bir_lowering=False, debug=False, num_devices=8)
    io = {}
    for name, dt in [("xT", FP16), ("xnat", FP16), ("kTa", FP16),
                     ("wq", FP16), ("wk", FP16), ("wo8", E4M3)]:
        io[name] = nc.dram_tensor(name, [E, E], dt, kind="ExternalInput").ap()
    io["vw8"] = nc.dram_tensor("vw8", [2 * E, E], E4M3, kind="ExternalInput").ap()
    io["bqk"] = nc.dram_tensor("bqk", [2, E], F32, kind="ExternalInput").ap()
    io["consts"] = nc.dram_tensor("consts", [1, 4 * E], FP16, kind="ExternalInput").ap()
    io["y16"] = nc.dram_tensor("y16", [L, E], FP16, kind="ExternalOutput").ap()
    io["w16"] = nc.dram_tensor("w16", [L, L], FP16, kind="ExternalOutput").ap()
    with tile.TileContext(nc) as tc:
        with ExitStack() as ctx:
            tc.ctx = ctx
            _emit(nc, tc, io)
    nc.compile()
    _CACHED = nc
    return nc


def kernel(query, key_t, value, in_proj_w, in_proj_b, out_proj_w, out_proj_b,
           ln_gamma, ln_beta, _trace=False, _tmpdir=None):
    query = np.asarray(query, dtype=np.float32)
    key_t = np.asarray(key_t, dtype=np.float32)
    value = np.asarray(value, dtype=np.float32)
    in_proj_w = np.asarray(in_proj_w, dtype=np.float32)
    out_proj_w = np.asarray(out_proj_w, dtype=np.float32)
    b = np.asarray(in_proj_b, dtype=np.float32)
    bo = np.asarray(out_proj_b, dtype=np.float32)

    wq = np.ascontiguousarray(in_proj_w[0:E].T).astype(NP_FP16)
    wk = np.ascontiguousarray(in_proj_w[E:2 * E].T).astype(NP_FP16)
    wv8 = np.ascontiguousarray(16.0 * in_proj_w[2 * E:3 * E].T).astype(NP_E4M3)
    wo8 = np.ascontiguousarray(16.0 * out_proj_w.T).astype(NP_E4M3)
    bqk = np.ascontiguousarray(np.stack([b[0:E], b[E:2 * E]]))
    consts = np.concatenate([
        16.0 * b[2 * E:3 * E], 256.0 * bo,
        np.asarray(ln_gamma, np.float32), np.asarray(ln_beta, np.float32),
    ]).reshape(1, 4 * E).astype(NP_FP16)

    nc = _build()
    in_maps = []
    for c in range(8):
        qc, kc, vc = query[c], key_t[c], value[c]
        vTa_ = np.ascontiguousarray(vc.T).astype(NP_E4M3)
        in_maps.append(dict(
            xT=np.ascontiguousarray(qc.T).astype(NP_FP16),
            xnat=qc.astype(NP_FP16),
            kTa=np.ascontiguousarray(kc.T).astype(NP_FP16),
            vw8=np.concatenate(sum(
                [[wv8[256 * g:256 * (g + 1)], vTa_[256 * g:256 * (g + 1)]]
                 for g in range(4)], [])),
            wq=wq, wk=wk, wo8=wo8, bqk=bqk, consts=consts,
        ))
    res = run_bass_kernel_spmd(
        nc, in_maps, core_ids=list(range(8)), trace=_trace, tmpdir=_tmpdir
    )
    y = np.stack([r["y16"].astype(np.float32) for r in res.results])
    w = np.stack([r["w16"].astype(np.float32) for r in res.results]) / float(H)
    kernel._last_result = res
    return y, w


# revision 3
# speedup vs baseline: 1.0193x; 1.0193x over previous
"""MultiHeadCrossAttention kernel for 8 Trainium2 NeuronCores.

Sharding: pure data-parallel over batch (B=8 -> 1 batch element per core).

Design (303,515ns cost-model estimate, from 310,945ns prior; rel err 2.6e-3):
  - Host pre-transposes/downcasts: fp16 q/k/scores/attn path, fp8-e4m3
    weights for v and out projections (x16 host scale, 1/256 residual STT).
  - v projection and out_proj in fp8 DoubleRow; q/k proj, scores, attn@v
    in fp16 on the PE.
  - Scores PSUM = THREE separate [P,2,512] tiles rotating per half-job:
    tile-granular dependency tracking gives exp->scores ~1.5 jobs of slack
    (a single multi-slot ring tile is tracked conservatively and locks the
    pipeline into a serial exp->scores->exp fixed point at ~3.8us/job).
  - softmax denominator via ones-column in v; reciprocal on DVE;
    partition-broadcast on Pool; exp on Act only (one act-table load).
  - attention-weights mean in fp16: normalize split DVE (kt 0-4) / Pool
    (kt 5-7), two parity accumulation chains on DVE; the per-q-block parity
    merge runs as a gpsimd-issued accumulate-DMA (software DGE) so it never
    occupies the DVE, then DMA-XBAR transposes to natural [q,k].
  - pipeline emitted b(oldest) -> a -> front so ready work never queues
    behind not-yet-ready work on the in-order engine queues; qk projection
    halves paced by a need-based counter so every kT/qT chunk is emitted
    before the first front that reads it (emission-order RAW requirement).
  - out_proj chunks drained two per b-step through the single proj psum
    bank; LayerNorm inline per q-block (bn_stats + rsqrt bit trick).
"""

import numpy as np
import ml_dtypes
from contextlib import ExitStack

import concourse.bacc as bacc
import concourse.bass as bass
import concourse.tile as tile
from concourse import mybir
from concourse.bass_utils import run_bass_kernel_spmd
from concourse.masks import make_identity

E = 1024
H = 16
DH = 64
L = 1024
P = 128
QB = 256          # q-block size
NQB = L // QB     # 4
NKT = L // P      # 8 k-tiles
NEC = E // P      # 8 feature chunks
VS = H * (DH + 1)  # 1040 v columns per k-tile (65 per head)
LN_EPS = 1e-5

F32 = mybir.dt.float32
FP16 = mybir.dt.float16
E4M3 = mybir.dt.float8e4
AF = mybir.ActivationFunctionType
OP = mybir.AluOpType
DR = mybir.MatmulPerfMode.DoubleRow

NP_FP16 = np.float16
NP_E4M3 = ml_dtypes.float8_e4m3


def _emit(nc, tc, io):
    ctx = tc.ctx
    ctx.enter_context(nc.allow_low_precision("fp8 attention"))

    const = ctx.enter_context(tc.tile_pool(name="const", bufs=1))
    persist = ctx.enter_context(tc.tile_pool(name="persist", bufs=1))

    ones1 = const.tile([1, P], FP16)
    nc.vector.memset(ones1[:], 1.0)
    one_u = const.tile([P, 1], mybir.dt.uint32)
    nc.vector.memset(one_u[:], 1)
    magic_u = const.tile([P, 1], mybir.dt.uint32)
    nc.vector.memset(magic_u[:], 0x5F3759DF)
    ident_f = const.tile([P, P], F32)
    make_identity(nc, ident_f[:])
    ident = const.tile([P, P], FP16)
    nc.vector.tensor_copy(ident[:], ident_f[:])
    ident8p = const.tile([P, 2, P], E4M3)
    nc.vector.tensor_copy(ident8p[:, 0, :], ident_f[:])
    nc.vector.tensor_copy(ident8p[:, 1, :], ident_f[:])

    # persistent activations / weights
    qT = persist.tile([P, NEC, L], FP16)     # [e%128, e//128, l]
    kT = persist.tile([P, NEC, L], FP16)
    v_sb = persist.tile([P, NKT * VS], FP16)  # [l%128, kt*(16 heads x 65)]
    wo8 = persist.tile([P, NEC, E], E4M3)    # 16*Wo.T  [e_in, e_out]

    # ones columns (softmax denominator trick)
    nc.vector.memset(
        v_sb[:].rearrange("p (n d) -> p n d", d=DH + 1)[:, :, DH:DH + 1], 1.0,
    )

    ld_pool = ctx.enter_context(tc.tile_pool(name="ld", bufs=1))
    # PSUM: manual slot management. sc 2 banks, av 1 bank, p1 1 bank, W 4.
    psum_all = ctx.enter_context(tc.tile_pool(name="psum", bufs=1, space="PSUM"))
    # 3 separate half-job score tiles: tile-granular dependency tracking gives
    # exp(m) -> scores(m+3) 1.5 jobs of real slack (single-tile rings are
    # tracked conservatively and serialize).
    sc_t = [psum_all.tile([P, 2, 512], F32, name=f"sc_t{i}")
            for i in range(3)]  # 6 banks
    av_sl = psum_all.tile([P, 2, QB], F32)     # 1 bank: 2 slots of [P,256]
    p1_sl = psum_all.tile([P, 512], F32)       # 1 bank: qk/out_proj/W bursts

    expT_pool = ctx.enter_context(tc.tile_pool(name="expT", bufs=6))
    wch_pool = ctx.enter_context(tc.tile_pool(name="wch", bufs=4))
    a8_pool = ctx.enter_context(tc.tile_pool(name="a8", bufs=2))
    invbc_pool = ctx.enter_context(tc.tile_pool(name="invbc", bufs=6))
    wacc_pool = ctx.enter_context(tc.tile_pool(name="wacc", bufs=1))
    wnat_pool = ctx.enter_context(tc.tile_pool(name="wnat", bufs=2))
    xqb_pool = ctx.enter_context(tc.tile_pool(name="xqb", bufs=2))
    small = ctx.enter_context(tc.tile_pool(name="small", bufs=4))
    z_pool = ctx.enter_context(tc.tile_pool(name="z16", bufs=2))
    ysb_pool = ctx.enter_context(tc.tile_pool(name="ysb", bufs=2))

    # ---- input loads, in dependency-criticality order ----
    vw = ld_pool.tile([P, 2 * NEC, E], E4M3, tag="aTx")
    aT_q = ld_pool.tile([P, NEC, L], FP16, tag="aTq")
    wt_q = ld_pool.tile([P, NEC, E], FP16, tag="wtq")
    # consts: one early DMA [1,4096] = [bv(1024)|bo(1024) | gamma | beta]
    crow = const.tile([1, 4 * E], FP16)
    nc.sync.dma_start(out=crow[:], in_=io["consts"][:])
    # vw8 arrives in 4 interleaved groups [w-pair | a-pair]
    for g in range(4):
        nc.sync.dma_start(
            out=vw[:, 4 * g:4 * (g + 1), :],
            in_=io["vw8"][512 * g:512 * (g + 1), :].rearrange(
                "(c p) n -> p c n", p=P),
        )
    bvo_row = crow[:, 0:2 * E]
    g_row = crow[:, 2 * E:3 * E]
    b_row = crow[:, 3 * E:4 * E]
    bqk_col = const.tile([P, 2 * NEC], F32)

    for i in range(2):
        nc.sync.dma_start(
            out=bqk_col[:, NEC * i:NEC * (i + 1)],
            in_=io["bqk"][i, :].rearrange("(m p) -> p m", p=P),
        )
    for g in range(4):
        nc.sync.dma_start(
            out=wt_q[:, 2 * g:2 * g + 2, :],
            in_=io["wq"][256 * g:256 * (g + 1), :].rearrange("(c p) n -> p c n", p=P))
        nc.sync.dma_start(
            out=aT_q[:, 2 * g:2 * g + 2, :],
            in_=io["xT"][256 * g:256 * (g + 1), :].rearrange("(c p) l -> p c l", p=P))
    gamma_bc = const.tile([P, E], FP16)
    beta_bc = const.tile([P, E], FP16)

    # ---- v projection (fp8 DoubleRow, x16 scale) ----
    # psum chains rotate through the scores ring (idle until jobs start) so
    # vproj pipelines against its Act drains and leaves p1 free for qk_proj
    for m in range(NEC):
        for n in range(2):
            i = 2 * m + n
            ps = sc_t[(i // 2) % 3][:, i % 2, :]
            for sub in range(2):
                for pr in range(4):
                    nc.tensor.matmul(
                        ps[:, 256 * sub:256 * (sub + 1)],
                        vw[:, 4 * pr + 2:4 * pr + 4, P * m:P * (m + 1)],
                        vw[:, 4 * pr:4 * pr + 2,
                           512 * n + 256 * sub:512 * n + 256 * (sub + 1)],
                        start=(pr == 0), stop=False,
                        perf_mode=DR,
                    )
                nc.tensor.matmul(
                    ps[:, 256 * sub:256 * (sub + 1)],
                    ones1[0:1, :],
                    bvo_row[:, 512 * n + 256 * sub:512 * n + 256 * (sub + 1)],
                    start=False, stop=True,
                )
            dst = v_sb[:, VS * m + 520 * n:VS * m + 520 * (n + 1)]
            nc.scalar.copy(
                out=dst.rearrange("p (h d) -> p h d", d=DH + 1)[:, :, 0:DH],
                in_=ps.rearrange("p (h d) -> p h d", d=DH),
            )

    nc.gpsimd.partition_broadcast(gamma_bc[:], g_row)
    nc.gpsimd.partition_broadcast(beta_bc[:], b_row)

    # k loads reuse the v buffers (freed by the v projection above)
    aT_k = ld_pool.tile([P, NEC, L], FP16, tag="aTx")
    wt_k = ld_pool.tile([P, NEC, E], FP16, tag="wtx")
    for g in range(4):
        nc.sync.dma_start(
            out=wt_k[:, 2 * g:2 * g + 2, :],
            in_=io["wk"][256 * g:256 * (g + 1), :].rearrange("(c p) n -> p c n", p=P))
        nc.sync.dma_start(
            out=aT_k[:, 2 * g:2 * g + 2, :],
            in_=io["kTa"][256 * g:256 * (g + 1), :].rearrange("(c p) l -> p c l", p=P))
    nc.sync.dma_start(out=wo8[:], in_=io["wo8"].rearrange("(c p) n -> p c n", p=P))

    def qk_proj_half(ti, m, n):
        aT, wt = (aT_q, wt_q) if ti == 0 else (aT_k, wt_k)
        ps = p1_sl[:]
        for c in range(NEC):
            nc.tensor.matmul(
                ps,
                wt[:, c, P * m:P * (m + 1)],
                aT[:, c, 512 * n:512 * (n + 1)],
                start=(c == 0), stop=(c == NEC - 1),
            )
        dst = (qT if ti == 0 else kT)[:, m, 512 * n:512 * (n + 1)]
        nc.scalar.activation(
            dst, ps, AF.Identity,
            bias=bqk_col[:, NEC * ti + m:NEC * ti + m + 1],
        )

    # ---- per-qb state ----
    st = {}

    def qb_begin(qb):
        q0 = QB * qb
        x_qb = xqb_pool.tile([P, 2, E], FP16, tag="xqb", name=f"xqb_{qb}")
        nc.sync.dma_start(
            out=x_qb[:],
            in_=io["xnat"][q0:q0 + QB, :].rearrange("(s p) e -> p s e", p=P),
        )
        st[qb] = dict(
            x_qb=x_qb,
            attnT8=a8_pool.tile([P, NEC, QB], E4M3, tag="attnT8", name=f"a8_{qb}"),
            ysb=ysb_pool.tile([P, 2, E], FP16, tag="ysb", name=f"y_{qb}"),
            Wacc=[wch_pool.tile([P, NKT * QB], FP16, tag="wch",
                                name=f"wa_{qb}_{p}") for p in range(2)],
        )

    qtr = [0]  # global quarter counter (ring position)

    def head_front(qb, h):
        if h == 0:
            qb_begin(qb)
        q0 = QB * qb
        hb = (h % 2) * DH
        hc = h // 2
        expT = expT_pool.tile([P, NKT * QB], FP16, tag="expT",
                              name=f"expT_{qb}_{h}")
        for half in range(2):
            stile = sc_t[qtr[0] % 3]
            qtr[0] += 1
            for j in range(4):
                kt = 4 * half + j
                nc.tensor.matmul(
                    stile[:, j // 2, 256 * (j % 2):256 * (j % 2 + 1)],
                    kT[hb:hb + DH, hc, P * kt:P * (kt + 1)],
                    qT[hb:hb + DH, hc, q0:q0 + QB],
                    start=True, stop=True,
                )
            nc.scalar.activation(
                expT[:, 1024 * half:1024 * (half + 1)],
                stile[:], AF.Exp, scale=0.125,
            )
        return expT

    def tail_a(qb, h, expT):
        s = st[qb]
        hb = (h % 2) * DH
        hc = h // 2
        av = av_sl[:, (H * qb + h) % 2, :]
        for kt in range(NKT):
            nc.tensor.matmul(
                av[0:DH + 1, 0:QB],
                v_sb[:, VS * kt + (DH + 1) * h:VS * kt + (DH + 1) * (h + 1)],
                expT[:, QB * kt:QB * (kt + 1)],
                start=(kt == 0), stop=(kt == NKT - 1),
            )
        inv = small.tile([1, QB], FP16, tag="inv", name=f"inv_{qb}_{h}")
        nc.vector.reciprocal(inv[:], av[DH:DH + 1, 0:QB])
        inv_bc = invbc_pool.tile([P, QB], FP16, tag="invbc", name=f"ib_{qb}_{h}")
        nc.gpsimd.partition_broadcast(inv_bc[:], inv[:])
        nc.vector.tensor_tensor(
            out=s["attnT8"][hb:hb + DH, hc, :],
            in0=av[0:DH, 0:QB], in1=inv_bc[0:DH, :], op=OP.mult,
        )
        return inv_bc

    def tail_b(qb, h, expT, inv_bc):
        s = st[qb]
        iap = inv_bc[:]
        KD = 5  # kt 0..KD-1 on DVE, rest on Pool
        ee = expT[:].rearrange("p (n d) -> p n d", d=QB)
        Wacc = s["Wacc"][h % 2]
        wa = Wacc[:].rearrange("p (n d) -> p n d", d=QB)
        if h <= 1:
            nc.vector.tensor_tensor(
                out=wa[:, 0:KD, :], in0=ee[:, 0:KD, :],
                in1=bass.AP(tensor=iap.tensor, offset=iap.offset,
                            ap=[iap.ap[0], [0, KD], iap.ap[1]]),
                op=OP.mult,
            )
            if KD < NKT:
                nc.gpsimd.tensor_tensor(
                    out=wa[:, KD:NKT, :], in0=ee[:, KD:NKT, :],
                    in1=bass.AP(tensor=iap.tensor, offset=iap.offset,
                                ap=[iap.ap[0], [0, NKT - KD], iap.ap[1]]),
                    op=OP.mult,
                )
        else:
            nc.vector.tensor_tensor(
                out=ee[:, 0:KD, :], in0=ee[:, 0:KD, :],
                in1=bass.AP(tensor=iap.tensor, offset=iap.offset,
                            ap=[iap.ap[0], [0, KD], iap.ap[1]]),
                op=OP.mult,
            )
            if KD < NKT:
                nc.gpsimd.tensor_tensor(
                    out=ee[:, KD:NKT, :], in0=ee[:, KD:NKT, :],
                    in1=bass.AP(tensor=iap.tensor, offset=iap.offset,
                                ap=[iap.ap[0], [0, NKT - KD], iap.ap[1]]),
                    op=OP.mult,
                )
            nc.vector.tensor_tensor(out=Wacc[:], in0=Wacc[:], in1=expT[:],
                                    op=OP.add)

    # out_proj for one psum chunk (qs, eb); p1 single slot, interleaved
    def op_chunk(qb, qs, eb):
        s = st[qb]
        po = p1_sl[:]
        attnT8 = s["attnT8"]
        for sub in range(2):
            for pr in range(4):
                nc.tensor.matmul(
                    po[:, 256 * sub:256 * (sub + 1)],
                    attnT8[:, 2 * pr:2 * pr + 2, P * qs:P * (qs + 1)],
                    wo8[:, 2 * pr:2 * pr + 2,
                        512 * eb + 256 * sub:512 * eb + 256 * (sub + 1)],
                    start=(pr == 0), stop=False,
                    perf_mode=DR,
                )
            nc.tensor.matmul(
                po[:, 256 * sub:256 * (sub + 1)],
                ones1[0:1, :],
                bvo_row[:, E + 512 * eb + 256 * sub:
                        E + 512 * eb + 256 * (sub + 1)],
                start=False, stop=True,
            )
        nc.vector.scalar_tensor_tensor(
            out=s["ysb"][:, qs, 512 * eb:512 * (eb + 1)],
            in0=po[:], scalar=1.0 / 256.0,
            in1=s["x_qb"][:, qs, 512 * eb:512 * (eb + 1)],
            op0=OP.mult, op1=OP.add,
        )

    def fw_merge(qb):
        W0, W1 = st[qb]["Wacc"]
        nc.gpsimd.dma_start(out=W0[:], in_=W1[:], accum_op=OP.add)

    def fw_out(qb, qs):
        q0 = QB * qb
        Wacc = st[qb]["Wacc"][0]
        wnat = wnat_pool.tile([P, NKT, P], FP16, tag="wnat",
                              name=f"wn_{qb}_{qs}")
        for kt in range(NKT):
            nc.sync.dma_start_transpose(
                wnat[:, kt, :],
                Wacc[:, QB * kt + P * qs:QB * kt + P * (qs + 1)],
            )
        nc.sync.dma_start(
            out=io["w16"][q0 + P * qs:q0 + P * (qs + 1), :], in_=wnat[:]
        )
        if qs == 1:
            st[qb]["Wacc"] = []

    def finalize_ln(qb):
        # LayerNorm: batched stats + one-shot rsqrt (bit trick + 1 Newton)
        yqb = st[qb]["ysb"]
        mvs = []
        for qs in range(2):
            t = 2 * qb + qs
            stats = small.tile([P, 2, 6], F32, tag="stats", name=f"st_{t}")
            ychg = yqb[:, qs, :].rearrange("p (s f) -> p s f", f=512)
            for sg in range(2):
                nc.vector.bn_stats(out=stats[:, sg, :], in_=ychg[:, sg, :])
            mv = small.tile([P, 2], F32, tag="mv", name=f"mv_{t}")
            nc.vector.bn_aggr(out=mv[:], in_=stats[:])
            mvs.append(mv)
        ve = small.tile([P, 2], F32, tag="ve", name=f"ve_{qb}")
        for qs in range(2):
            nc.vector.tensor_scalar_add(out=ve[:, qs:qs + 1],
                                        in0=mvs[qs][:, 1:2], scalar1=LN_EPS)
        y0u = small.tile([P, 2], mybir.dt.uint32, tag="y0u", name=f"y0_{qb}")
        ou = bass.AP(tensor=one_u.tensor, offset=one_u[:].offset,
                     ap=[one_u[:].ap[0], [0, 2]])
        mu = bass.AP(tensor=magic_u.tensor, offset=magic_u[:].offset,
                     ap=[magic_u[:].ap[0], [0, 2]])
        nc.vector.tensor_tensor(out=y0u[:], in0=ve[:].bitcast(mybir.dt.uint32),
                                in1=ou, op=OP.logical_shift_right)
        nc.vector.tensor_tensor(out=y0u[:], in0=mu, in1=y0u[:], op=OP.subtract)
        y0 = y0u[:].bitcast(F32)
        rstd = small.tile([P, 2], F32, tag="rstd", name=f"rs_{qb}")
        tmp = small.tile([P, 2], F32, tag="tmp", name=f"tm_{qb}")
        nc.vector.tensor_tensor(out=tmp[:], in0=y0, in1=y0, op=OP.mult)
        nc.vector.tensor_tensor(out=tmp[:], in0=tmp[:], in1=ve[:], op=OP.mult)
        nc.vector.tensor_scalar(out=tmp[:], in0=tmp[:], scalar1=-0.5,
                                scalar2=1.5, op0=OP.mult, op1=OP.add)
        nc.vector.tensor_tensor(out=rstd[:], in0=y0, in1=tmp[:], op=OP.mult)
        for qs in range(2):
            t = 2 * qb + qs
            negmr = small.tile([P, 1], F32, tag="negmr", name=f"nm_{t}")
            nc.vector.tensor_tensor(out=negmr[:], in0=mvs[qs][:, 0:1],
                                    in1=rstd[:, qs:qs + 1], op=OP.mult)
            nc.vector.tensor_scalar_mul(out=negmr[:], in0=negmr[:], scalar1=-1.0)
            z16 = z_pool.tile([P, E], FP16, tag="z16", name=f"z_{t}")
            nc.scalar.activation(z16[:], yqb[:, qs, :], AF.Identity,
                                 bias=negmr[:], scale=rstd[:, qs:qs + 1])
            nc.vector.tensor_tensor(out=z16[:], in0=z16[:], in1=gamma_bc[:],
                                    op=OP.mult)
            nc.vector.tensor_tensor(out=z16[:], in0=z16[:], in1=beta_bc[:],
                                    op=OP.add)
            nc.sync.dma_start(out=io["y16"][P * t:P * (t + 1), :], in_=z16[:])

    # ---- fused pipeline ----
    # qk halves queue: q-proj m0..2 first (6 halves), then interleave rest
    qk_queue = [(0, m, n) for m in range(3) for n in range(2)]
    qk_rest = []
    for m in range(NEC):
        qk_rest.append((1, m, 0))
        qk_rest.append((1, m, 1))
        if m + 3 < NEC:
            qk_rest.append((0, m + 3, 0))
            qk_rest.append((0, m + 3, 1))
    op_queue = []   # pending out_proj chunks

    for t in qk_queue:
        qk_proj_half(*t)

    jobs = [(qb, h) for qb in range(NQB) for h in range(H)]
    N = len(jobs)
    fronts = {}
    invs = {}
    nf = 0
    na = 0
    nb = 0

    def emit_front():
        nonlocal nf
        qb, h = jobs[nf]
        fronts[nf] = head_front(qb, h)
        nf += 1

    def emit_a():
        nonlocal na
        qb, h = jobs[na]
        invs[na] = tail_a(qb, h, fronts[na])
        na += 1
        if h == H - 1:
            for qs in range(2):
                for eb in range(2):
                    op_queue.append((qb, qs, eb))

    def emit_b():
        nonlocal nb
        qb, h = jobs[nb]
        tail_b(qb, h, fronts.pop(nb), invs.pop(nb))
        nb += 1
        if h == H - 1:
            fw_merge(qb)
        elif h == 0 and qb > 0:
            fw_out(qb - 1, 0)
        elif h == 1 and qb > 0:
            fw_out(qb - 1, 1)
        # drain up to 2 out_proj chunks per b-step
        for _ in range(2):
            if op_queue:
                op_chunk(*op_queue.pop(0))

    ln_done = set()

    def maybe_ln():
        # LN(qb) once all 4 out_proj chunks for qb are emitted
        for qb in range(NQB):
            if qb in ln_done:
                continue
            if all((qb, qs, eb) not in op_queue
                   for qs in range(2) for eb in range(2)) \
                    and nb >= H * (qb + 1) + 6:
                ln_done.add(qb)
                finalize_ln(qb)

    # steady pipeline; b (oldest, deps ready) before a before fronts so
    # ready work never queues behind not-yet-ready work on DVE/Pool/PE
    qi = 0
    while nb < N:
        # qk halves must stay ahead of the fronts that read them: fronts for
        # jobs 2i,2i+1 (qb0) need k chunk m=i and q chunk m=i (m>=3 from rest)
        need = 4 * (nf // 2 + 2)
        if nb < na - 1 or (na == N and nb < N):
            emit_b()
        if qi < min(need, len(qk_rest)):
            qk_proj_half(*qk_rest[qi])
            qi += 1
        if nb < na - 1 or (na == N and nb < N):
            emit_b()
        if qi < min(need, len(qk_rest)):
            qk_proj_half(*qk_rest[qi])
            qi += 1
        if na < nf - 1 or (nf == N and na < N):
            emit_a()
        if na < nf - 1 or (nf == N and na < N):
            emit_a()
        if qi < min(need, len(qk_rest)):
            qk_proj_half(*qk_rest[qi])
            qi += 1
        if nf < N:
            emit_front()
        if qi < min(need, len(qk_rest)):
            qk_proj_half(*qk_rest[qi])
            qi += 1
        if nf < N:
            emit_front()
        maybe_ln()
    # tail: release the y16 path (out_proj + LN) early, then the qb3 W quads
    while op_queue:
        op_chunk(*op_queue.pop(0))
    maybe_ln()
    for qb in range(NQB):
        if qb not in ln_done:
            finalize_ln(qb)
    fw_out(NQB - 1, 0)
    fw_out(NQB - 1, 1)


_CACHED = None


def _build():
    global _CACHED
    if _CACHED is not None:
        return _CACHED
    nc = bacc.Bacc("TRN2", target_bir_lowering=False, debug=False, num_devices=8)
    io = {}
    for name, dt in [("xT", FP16), ("xnat", FP16), ("kTa", FP16),
                     ("wq", FP16), ("wk", FP16), ("wo8", E4M3)]:
        io[name] = nc.dram_tensor(name, [E, E], dt, kind="ExternalInput").ap()
    io["vw8"] = nc.dram_tensor("vw8", [2 * E, E], E4M3, kind="ExternalInput").ap()
    io["bqk"] = nc.dram_tensor("bqk", [2, E], F32, kind="ExternalInput").ap()
    io["consts"] = nc.dram_tensor("consts", [1, 4 * E], FP16, kind="ExternalInput").ap()
    io["y16"] = nc.dram_tensor("y16", [L, E], FP16, kind="ExternalOutput").ap()
    io["w16"] = nc.dram_tensor("w16", [L, L], FP16, kind="ExternalOutput").ap()
    with tile.TileContext(nc) as tc:
        with ExitStack() as ctx:
            tc.ctx = ctx
            _emit(nc, tc, io)
    nc.compile()
    _CACHED = nc
    return nc


def kernel(query, key_t, value, in_proj_w, in_proj_b, out_proj_w, out_proj_b,
           ln_gamma, ln_beta, _trace=False, _tmpdir=None):
    query = np.asarray(query, dtype=np.float32)
    key_t = np.asarray(key_t, dtype=np.float32)
    value = np.asarray(value, dtype=np.float32)
    in_proj_w = np.asarray(in_proj_w, dtype=np.float32)
    out_proj_w = np.asarray(out_proj_w, dtype=np.float32)
    b = np.asarray(in_proj_b, dtype=np.float32)
    bo = np.asarray(out_proj_b, dtype=np.float32)

    wq = np.ascontiguousarray(in_proj_w[0:E].T).astype(NP_FP16)
    wk = np.ascontiguousarray(in_proj_w[E:2 * E].T).astype(NP_FP16)
    wv8 = np.ascontiguousarray(16.0 * in_proj_w[2 * E:3 * E].T).astype(NP_E4M3)
    wo8 = np.ascontiguousarray(16.0 * out_proj_w.T).astype(NP_E4M3)
    bqk = np.ascontiguousarray(np.stack([b[0:E], b[E:2 * E]]))
    consts = np.concatenate([
        16.0 * b[2 * E:3 * E], 256.0 * bo,
        np.asarray(ln_gamma, np.float32), np.asarray(ln_beta, np.float32),
    ]).reshape(1, 4 * E).astype(NP_FP16)

    nc = _build()
    in_maps = []
    for c in range(8):
        qc, kc, vc = query[c], key_t[c], value[c]
        vTa_ = np.ascontiguousarray(vc.T).astype(NP_E4M3)
        in_maps.append(dict(
            xT=np.ascontiguousarray(qc.T).astype(NP_FP16),
            xnat=qc.astype(NP_FP16),
            kTa=np.ascontiguousarray(kc.T).astype(NP_FP16),
            vw8=np.concatenate(sum(
                [[wv8[256 * g:256 * (g + 1)], vTa_[256 * g:256 * (g + 1)]]
                 for g in range(4)], [])),
            wq=wq, wk=wk, wo8=wo8, bqk=bqk, consts=consts,
        ))
    res = run_bass_kernel_spmd(
        nc, in_maps, core_ids=list(range(8)), trace=_trace, tmpdir=_tmpdir
    )
    y = np.stack([r["y16"].astype(np.float32) for r in res.results])
    w = np.stack([r["w16"].astype(np.float32) for r in res.results]) / float(H)
    kernel._last_result = res
    return y, w
